# revision 5
# baseline (speedup 1.0000x reference)
"""Trainium2 Bass kernel for nn_AttentionBlock (GroupNorm + MHSA + proj + residual).

Data-parallel over batch: B=8 batch elements -> 8 NeuronCores, one each.
Per core (all in fp32, matmuls in fp32r):
  x_b [C=512, N=1024]
  group_norm (32 groups of 16 channels)
  qkv = W_qkv @ x_norm + b  (q,k in [c,n] layout; v computed directly transposed [n,c])
  per head h (8 heads, hd=64): S^T = k_h^T q_h  [m,n]; P = exp(S^T/8);
  out_h = v_h @ P / colsum (colsum via ones-column appended to v^T -> M=65 matmul)
  y = x + W_proj @ out + b_proj
"""

import numpy as np

C = 512
N = 1024  # H*W
NH = 8
HD = 64
NG = 32
EPS = 1e-5
NCORES = 8

_CACHE = {}


def _build_program():
    import concourse.bass as bass  # noqa: F401
    import concourse.mybir as mybir
    import concourse.tile as tile
    from concourse import bacc

    f32 = mybir.dt.float32
    f32r = mybir.dt.float32r
    Act = mybir.ActivationFunctionType
    from concourse.alu_op_type import AluOpType as Op

    nc = bacc.Bacc("TRN2", target_bir_lowering=False, debug=False, num_devices=NCORES)

    # DRAM parameters (per-core shapes; same weights on all cores, x sharded by batch)
    x_d = nc.dram_tensor("x", [C, N], f32, kind="ExternalInput")
    wqkvT_d = nc.dram_tensor("wqkvT", [C, 3 * C], f32r, kind="ExternalInput")
    bqkv_d = nc.dram_tensor("bqkv", [1, 3 * C], f32r, kind="ExternalInput")
    wprojT_d = nc.dram_tensor("wprojT", [C, C], f32r, kind="ExternalInput")
    bproj_d = nc.dram_tensor("bproj", [1, C], f32r, kind="ExternalInput")
    gammaT_d = nc.dram_tensor("gammaT", [128, 4], f32, kind="ExternalInput")
    betaT_d = nc.dram_tensor("betaT", [128, 4], f32, kind="ExternalInput")
    gsel_d = nc.dram_tensor("gsel", [128, 4, NG], f32, kind="ExternalInput")
    bsel_d = nc.dram_tensor("bsel", [NG, C], f32, kind="ExternalInput")
    onesr_d = nc.dram_tensor("onesr", [1, 512], f32r, kind="ExternalInput")
    onescol_d = nc.dram_tensor("onescol", [128, NH], f32r, kind="ExternalInput")
    y_d = nc.dram_tensor("y", [C, N], f32, kind="ExternalOutput")

    with tile.TileContext(nc) as tc:
        with tc.tile_pool(name="mem", bufs=1) as mem:
            # ---- persistent tiles ----
            x_t = [mem.tile([128, N], f32, tag=f"x{k}", name=f"x{k}") for k in range(4)]
            xn_t = [mem.tile([128, N], f32r, tag=f"xn{k}", name=f"xn{k}") for k in range(4)]
            wq_t = [mem.tile([128, 3 * C], f32r, tag=f"wq{k}", name=f"wq{k}") for k in range(4)]
            wp_t = [mem.tile([128, C], f32r, tag=f"wp{k}", name=f"wp{k}") for k in range(4)]
            bq_t = mem.tile([1, 3 * C], f32r, tag="bq", name="bq")
            bp_t = mem.tile([1, C], f32r, tag="bp", name="bp")
            gam_t = mem.tile([128, 4], f32, tag="gam", name="gam")
            bet_t = mem.tile([128, 4], f32, tag="bet", name="bet")
            gsel_t = mem.tile([128, 4, NG], f32, tag="gsel", name="gsel")
            bsel_t = mem.tile([NG, C], f32, tag="bsel", name="bsel")
            ones_t = mem.tile([1, 512], f32r, tag="ones", name="ones")
            qk_t = [mem.tile([128, N], f32r, tag=f"qk{oc}", name=f"qk{oc}") for oc in range(8)]
            # v^T with a ones column appended per head: [128, 8 heads, 65]
            vp_t = [mem.tile([128, NH, HD + 1], f32r, tag=f"vp{mc}", name=f"vp{mc}") for mc in range(8)]
            out_t = [mem.tile([128, N], f32r, tag=f"out{hp}", name=f"out{hp}") for hp in range(4)]
            y_t = [mem.tile([128, N], f32, tag=f"y{k}", name=f"y{k}") for k in range(4)]

            for k in range(4):
                nc.sync.dma_start(out=x_t[k], in_=x_d[k * 128:(k + 1) * 128, :])
                nc.sync.dma_start(out=wq_t[k], in_=wqkvT_d[k * 128:(k + 1) * 128, :])
                nc.sync.dma_start(out=wp_t[k], in_=wprojT_d[k * 128:(k + 1) * 128, :])
            nc.sync.dma_start(out=bq_t, in_=bqkv_d[:])
            nc.sync.dma_start(out=bp_t, in_=bproj_d[:])
            nc.sync.dma_start(out=gam_t, in_=gammaT_d[:])
            nc.sync.dma_start(out=bet_t, in_=betaT_d[:])
            nc.sync.dma_start(out=gsel_t, in_=gsel_d[:])
            nc.sync.dma_start(out=bsel_t, in_=bsel_d[:])
            nc.sync.dma_start(out=ones_t, in_=onesr_d[:])
            for mc in range(8):
                nc.sync.dma_start(out=vp_t[mc][:, :, HD:HD + 1], in_=onescol_d[:, :, None])

            # ---- group norm ----
            with (
                tc.tile_pool(name="gn", bufs=1) as gn,
                tc.tile_pool(name="pgn", bufs=2, space="PSUM") as pgn,
            ):
                s2_t = []
                for k in range(4):
                    st = gn.tile([128, 2, 6], f32, tag=f"st{k}", name=f"st{k}")
                    for j in range(2):
                        nc.vector.bn_stats(out=st[:, j, :], in_=x_t[k][:, j * 512:(j + 1) * 512])
                    mv = gn.tile([128, 2], f32, tag=f"mv{k}", name=f"mv{k}")
                    nc.vector.bn_aggr(out=mv, in_=st)
                    s2 = gn.tile([128, 2], f32, tag=f"s2{k}", name=f"s2{k}")
                    nc.vector.tensor_copy(out=s2[:, 0:1], in_=mv[:, 0:1])
                    nc.vector.tensor_tensor(out=s2[:, 1:2], in0=mv[:, 0:1], in1=mv[:, 0:1], op=Op.mult)
                    nc.vector.tensor_tensor(out=s2[:, 1:2], in0=s2[:, 1:2], in1=mv[:, 1:2], op=Op.add)
                    s2_t.append(s2)
                mvps = pgn.tile([NG, 2], f32, tag="mvps", name="mvps")
                for k in range(4):
                    nc.tensor.matmul(mvps, gsel_t[:, k, :], s2_t[k], start=(k == 0), stop=(k == 3))
                gn2 = gn.tile([NG, 2], f32, tag="gn2", name="gn2")
                eps_t = gn.tile([NG, 1], f32, tag="eps", name="eps")
                nc.vector.memset(eps_t, EPS)
                nc.vector.tensor_copy(out=gn2, in_=mvps)
                gnv = gn.tile([NG, 1], f32, tag="gnv", name="gnv")
                nc.vector.tensor_tensor(out=gnv, in0=gn2[:, 0:1], in1=gn2[:, 0:1], op=Op.mult)
                nc.vector.tensor_tensor(out=gn2[:, 1:2], in0=gn2[:, 1:2], in1=gnv, op=Op.subtract)
                nc.scalar.activation(out=gn2[:, 1:2], in_=gn2[:, 1:2], func=Act.Sqrt, bias=eps_t, scale=1.0)
                nc.vector.reciprocal(out=gn2[:, 1:2], in_=gn2[:, 1:2])
                for k in range(4):
                    bcp = pgn.tile([128, 2], f32, tag="bcp", name="bcp")
                    nc.tensor.matmul(bcp, bsel_t[:, k * 128:(k + 1) * 128], gn2, start=True, stop=True)
                    sc = gn.tile([128, 1], f32, tag=f"sc{k}", name=f"sc{k}")
                    tcv = gn.tile([128, 1], f32, tag=f"tc{k}", name=f"tc{k}")
                    nc.vector.tensor_tensor(out=sc, in0=bcp[:, 1:2], in1=gam_t[:, k:k + 1], op=Op.mult)
                    nc.vector.tensor_tensor(out=tcv, in0=bcp[:, 0:1], in1=sc, op=Op.mult)
                    nc.vector.tensor_tensor(out=tcv, in0=bet_t[:, k:k + 1], in1=tcv, op=Op.subtract)
                    nc.vector.tensor_scalar(out=xn_t[k], in0=x_t[k], scalar1=sc, scalar2=tcv,
                                            op0=Op.mult, op1=Op.add)

            # ---- QKV ----
            with tc.tile_pool(name="pqkv", bufs=3, space="PSUM") as pqkv:
                # q, k in standard [c, n] layout (o-chunks 0..7 of 3C)
                for oc in range(8):
                    osl = slice(oc * 128, (oc + 1) * 128)
                    for nt in range(2):
                        nsl = slice(nt * 512, (nt + 1) * 512)
                        ps = pqkv.tile([128, 512], f32, tag="qkv", name="qkv")
                        nc.tensor.matmul(ps, bq_t[0:1, osl], ones_t, start=True, stop=False)
                        for k in range(4):
                            nc.tensor.matmul(ps, wq_t[k][:, osl], xn_t[k][:, nsl],
                                             start=False, stop=(k == 3))
                        nc.vector.tensor_copy(out=qk_t[oc][:, nsl], in_=ps)
                # v^T in [m, c] layout (m-chunks 0..7), bias broadcast along m
                for mc in range(8):
                    msl = slice(mc * 128, (mc + 1) * 128)
                    ps = pqkv.tile([128, 512], f32, tag="qkv", name="qkv")
                    nc.tensor.matmul(ps, ones_t[0:1, 0:128], bq_t[0:1, 2 * C:3 * C], start=True, stop=False)
                    for k in range(4):
                        nc.tensor.matmul(ps, xn_t[k][:, msl], wq_t[k][:, 2 * C:3 * C],
                                         start=False, stop=(k == 3))
                    nc.vector.tensor_copy(
                        out=vp_t[mc][:, :, 0:HD],
                        in_=ps.rearrange("p (h c) -> p h c", h=NH),
                    )

            # ---- attention ----
            with (
                tc.tile_pool(name="att", bufs=2) as att,
                tc.tile_pool(name="pS", bufs=2, space="PSUM") as pS,
                tc.tile_pool(name="pO", bufs=2, space="PSUM") as pO,
            ):
                for hp in range(4):
                    q_t = qk_t[hp]
                    k_t = qk_t[4 + hp]
                    for nt in range(2):
                        nsl = slice(nt * 512, (nt + 1) * 512)
                        outA = pO.tile([HD + 1, 512], f32, tag="outA", name="outA")
                        outB = pO.tile([HD + 1, 512], f32, tag="outB", name="outB")
                        for mc in range(8):
                            msl = slice(mc * 128, (mc + 1) * 128)
                            Sps = pS.tile([128, 1024], f32, tag="S", name="S")
                            nc.tensor.matmul(Sps[:, 0:512], k_t[0:64, msl], q_t[0:64, nsl],
                                             start=True, stop=True)
                            nc.tensor.matmul(Sps[:, 512:1024], k_t[64:128, msl], q_t[64:128, nsl],
                                             start=True, stop=True, tile_position=(64, 0))
                            ex = att.tile([128, 1024], f32r, tag="ex", name="ex")
                            nc.scalar.activation(out=ex, in_=Sps, func=Act.Exp, scale=0.125)
                            nc.tensor.matmul(outA, vp_t[mc][:, 2 * hp, :], ex[:, 0:512],
                                             start=(mc == 0), stop=(mc == 7))
                            nc.tensor.matmul(outB, vp_t[mc][:, 2 * hp + 1, :], ex[:, 512:1024],
                                             start=(mc == 0), stop=(mc == 7))
                        # normalize: row 64 of outA/outB hold the softmax denominators
                        rcA = att.tile([65, 512], f32, tag="rcA", name="rcA")
                        rcB = att.tile([65, 512], f32, tag="rcB", name="rcB")
                        nc.vector.reciprocal(out=rcA[64:65, :], in_=outA[64:65, :])
                        nc.vector.reciprocal(out=rcB[64:65, :], in_=outB[64:65, :])
                        rc2 = att.tile([1, 1024], f32, tag="rc2", name="rc2")
                        nc.sync.dma_start(out=rc2[0:1, 0:512], in_=rcA[64:65, :])
                        nc.sync.dma_start(out=rc2[0:1, 512:1024], in_=rcB[64:65, :])
                        bc2 = att.tile([64, 1024], f32, tag="bc2", name="bc2")
                        nc.gpsimd.partition_broadcast(bc2, rc2[0:1, :])
                        nc.vector.tensor_tensor(out=out_t[hp][0:64, nsl], in0=outA[0:64, :],
                                                in1=bc2[:, 0:512], op=Op.mult)
                        stagB = att.tile([64, 512], f32r, tag="stagB", name="stagB")
                        nc.vector.tensor_tensor(out=stagB, in0=outB[0:64, :],
                                                in1=bc2[:, 512:1024], op=Op.mult)
                        nc.sync.dma_start(out=out_t[hp][64:128, nsl], in_=stagB)

            # ---- proj + residual ----
            with tc.tile_pool(name="pprj", bufs=3, space="PSUM") as pprj:
                for oc in range(4):
                    osl = slice(oc * 128, (oc + 1) * 128)
                    for nt in range(2):
                        nsl = slice(nt * 512, (nt + 1) * 512)
                        ps = pprj.tile([128, 512], f32, tag="prj", name="prj")
                        nc.tensor.matmul(ps, bp_t[0:1, osl], ones_t, start=True, stop=False)
                        for k in range(4):
                            nc.tensor.matmul(ps, wp_t[k][:, osl], out_t[k][:, nsl],
                                             start=False, stop=(k == 3))
                        nc.vector.tensor_tensor(out=y_t[oc][:, nsl], in0=ps, in1=x_t[oc][:, nsl],
                                                op=Op.add)
                for oc in range(4):
                    nc.sync.dma_start(out=y_d[oc * 128:(oc + 1) * 128, :], in_=y_t[oc])

    nc.compile()
    return nc


def _host_inputs(x, gamma, beta, w_qkv, b_qkv, w_proj, b_proj):
    f = np.float32
    xb = np.ascontiguousarray(np.asarray(x, f).reshape(NCORES, C, N))
    wqkvT = np.ascontiguousarray(np.asarray(w_qkv, f).T)          # [C, 3C]
    bq = np.ascontiguousarray(np.asarray(b_qkv, f)[None, :])      # [1, 3C]
    wprojT = np.ascontiguousarray(np.asarray(w_proj, f).T)        # [C, C]
    bp = np.ascontiguousarray(np.asarray(b_proj, f)[None, :])     # [1, C]
    gT = np.ascontiguousarray(np.asarray(gamma, f).reshape(4, 128).T)  # [128, 4]
    bT = np.ascontiguousarray(np.asarray(beta, f).reshape(4, 128).T)
    gsel = np.zeros((128, 4, NG), f)
    bsel = np.zeros((NG, C), f)
    for k in range(4):
        for p in range(128):
            g = 8 * k + p // 16
            gsel[p, k, g] = 1.0 / 16.0
            bsel[g, k * 128 + p] = 1.0
    onesr = np.ones((1, 512), f)
    onescol = np.ones((128, NH), f)
    shared = {"wqkvT": wqkvT, "bqkv": bq, "wprojT": wprojT, "bproj": bp,
              "gammaT": gT, "betaT": bT, "gsel": gsel, "bsel": bsel,
              "onesr": onesr, "onescol": onescol}
    return [dict(shared, x=xb[i]) for i in range(NCORES)]


def run(inputs, trace=False, **kwargs):
    from concourse.bass_utils import run_bass_kernel_spmd
    if "nc" not in _CACHE:
        _CACHE["nc"] = _build_program()
    nc = _CACHE["nc"]
    in_maps = _host_inputs(**inputs)
    res = run_bass_kernel_spmd(nc, in_maps, core_ids=list(range(NCORES)), trace=trace, **kwargs)
    B = inputs["x"].shape[0]
    H = W = 32
    y = np.stack([res.results[i]["y"].reshape(C, H, W) for i in range(NCORES)])
    return y.astype(np.float32), res


def kernel(**inputs):
    y, _ = run(inputs, trace=False)
    return y


# revision 7
# speedup vs baseline: 1.2567x; 1.2567x over previous
"""Trainium2 Bass kernel for nn_AttentionBlock (GroupNorm + MHSA + proj + residual).

Data-parallel over batch: B=8 batch elements -> 8 NeuronCores, one each.
Per core (all in fp32, matmuls in fp32r):
  x_b [C=512, N=1024]
  group_norm (32 groups of 16 channels)
  qkv = W_qkv @ x_norm + b  (q,k in [c,n] layout; v computed directly transposed [n,c])
  per head h (8 heads, hd=64): S^T = k_h^T q_h  [m,n]; P = exp(S^T/8);
  out_h = v_h @ P / colsum (colsum via ones-column appended to v^T -> M=65 matmul)
  y = x + W_proj @ out + b_proj
"""

import numpy as np

C = 512
N = 1024  # H*W
NH = 8
HD = 64
NG = 32
EPS = 1e-5
NCORES = 8

_CACHE = {}


def _build_program():
    import concourse.bass as bass  # noqa: F401
    import concourse.mybir as mybir
    import concourse.tile as tile
    from concourse import bacc

    f32 = mybir.dt.float32
    bf16 = mybir.dt.bfloat16
    Act = mybir.ActivationFunctionType
    from concourse.alu_op_type import AluOpType as Op

    nc = bacc.Bacc("TRN2", target_bir_lowering=False, debug=False, num_devices=NCORES)

    # DRAM parameters (per-core shapes; same weights on all cores, x sharded by batch)
    x_d = nc.dram_tensor("x", [C, N], f32, kind="ExternalInput")
    wqkvT_d = nc.dram_tensor("wqkvT", [C, 3 * C], bf16, kind="ExternalInput")
    bqkv_d = nc.dram_tensor("bqkv", [1, 3 * C], bf16, kind="ExternalInput")
    wprojT_d = nc.dram_tensor("wprojT", [C, C], bf16, kind="ExternalInput")
    bproj_d = nc.dram_tensor("bproj", [1, C], bf16, kind="ExternalInput")
    gammaT_d = nc.dram_tensor("gammaT", [128, 4], f32, kind="ExternalInput")
    betaT_d = nc.dram_tensor("betaT", [128, 4], f32, kind="ExternalInput")
    gsel_d = nc.dram_tensor("gsel", [128, 4, NG], f32, kind="ExternalInput")
    bsel_d = nc.dram_tensor("bsel", [NG, C], f32, kind="ExternalInput")
    onesr_d = nc.dram_tensor("onesr", [1, 512], bf16, kind="ExternalInput")
    onescol_d = nc.dram_tensor("onescol", [128, NH], bf16, kind="ExternalInput")
    y_d = nc.dram_tensor("y", [C, N], f32, kind="ExternalOutput")

    with tile.TileContext(nc) as tc:
        with tc.tile_pool(name="mem", bufs=1) as mem:
            # ---- persistent tiles ----
            x_t = [mem.tile([128, N], f32, tag=f"x{k}", name=f"x{k}") for k in range(4)]
            xn_t = [mem.tile([128, N], bf16, tag=f"xn{k}", name=f"xn{k}") for k in range(4)]
            wq_t = [mem.tile([128, 3 * C], bf16, tag=f"wq{k}", name=f"wq{k}") for k in range(4)]
            wp_t = [mem.tile([128, C], bf16, tag=f"wp{k}", name=f"wp{k}") for k in range(4)]
            bq_t = mem.tile([1, 3 * C], bf16, tag="bq", name="bq")
            bp_t = mem.tile([1, C], bf16, tag="bp", name="bp")
            gam_t = mem.tile([128, 4], f32, tag="gam", name="gam")
            bet_t = mem.tile([128, 4], f32, tag="bet", name="bet")
            gsel_t = mem.tile([128, 4, NG], f32, tag="gsel", name="gsel")
            bsel_t = mem.tile([NG, C], f32, tag="bsel", name="bsel")
            ones_t = mem.tile([1, 512], bf16, tag="ones", name="ones")
            qk_t = [mem.tile([128, N], bf16, tag=f"qk{oc}", name=f"qk{oc}") for oc in range(8)]
            # v^T with a ones column appended per head: [128, 8 heads, 65]
            vp_t = [mem.tile([128, NH, HD + 1], bf16, tag=f"vp{mc}", name=f"vp{mc}") for mc in range(8)]
            out_t = [mem.tile([128, N], bf16, tag=f"out{hp}", name=f"out{hp}") for hp in range(4)]
            y_t = [mem.tile([128, N], f32, tag=f"y{k}", name=f"y{k}") for k in range(4)]

            for k in range(4):
                nc.sync.dma_start(out=x_t[k], in_=x_d[k * 128:(k + 1) * 128, :])
            nc.gpsimd.dma_start(out=gam_t, in_=gammaT_d[:])
            nc.gpsimd.dma_start(out=bet_t, in_=betaT_d[:])
            nc.gpsimd.dma_start(out=gsel_t, in_=gsel_d[:])
            nc.gpsimd.dma_start(out=bsel_t, in_=bsel_d[:])
            nc.gpsimd.dma_start(out=ones_t, in_=onesr_d[:])
            nc.gpsimd.dma_start(out=bq_t, in_=bqkv_d[:])
            nc.gpsimd.dma_start(out=bp_t, in_=bproj_d[:])
            for k in range(4):
                nc.sync.dma_start(out=wq_t[k], in_=wqkvT_d[k * 128:(k + 1) * 128, :])
            for k in range(4):
                nc.sync.dma_start(out=wp_t[k], in_=wprojT_d[k * 128:(k + 1) * 128, :])
            for mc in range(8):
                nc.gpsimd.dma_start(out=vp_t[mc][:, :, HD:HD + 1], in_=onescol_d[:, :, None])

            # ---- group norm ----
            with (
                tc.tile_pool(name="gn", bufs=1) as gn,
                tc.tile_pool(name="pgn", bufs=2, space="PSUM") as pgn,
            ):
                s2_t = []
                for k in range(4):
                    st = gn.tile([128, 2, 6], f32, tag=f"st{k}", name=f"st{k}")
                    for j in range(2):
                        nc.vector.bn_stats(out=st[:, j, :], in_=x_t[k][:, j * 512:(j + 1) * 512])
                    mv = gn.tile([128, 2], f32, tag=f"mv{k}", name=f"mv{k}")
                    nc.vector.bn_aggr(out=mv, in_=st)
                    s2 = gn.tile([128, 2], f32, tag=f"s2{k}", name=f"s2{k}")
                    nc.vector.tensor_copy(out=s2[:, 0:1], in_=mv[:, 0:1])
                    nc.vector.tensor_tensor(out=s2[:, 1:2], in0=mv[:, 0:1], in1=mv[:, 0:1], op=Op.mult)
                    nc.vector.tensor_tensor(out=s2[:, 1:2], in0=s2[:, 1:2], in1=mv[:, 1:2], op=Op.add)
                    s2_t.append(s2)
                mvps = pgn.tile([NG, 2], f32, tag="mvps", name="mvps")
                for k in range(4):
                    nc.tensor.matmul(mvps, gsel_t[:, k, :], s2_t[k], start=(k == 0), stop=(k == 3))
                gn2 = gn.tile([NG, 2], f32, tag="gn2", name="gn2")
                eps_t = gn.tile([NG, 1], f32, tag="eps", name="eps")
                nc.vector.memset(eps_t, EPS)
                nc.vector.tensor_copy(out=gn2, in_=mvps)
                gnv = gn.tile([NG, 1], f32, tag="gnv", name="gnv")
                nc.vector.tensor_tensor(out=gnv, in0=gn2[:, 0:1], in1=gn2[:, 0:1], op=Op.mult)
                nc.vector.tensor_tensor(out=gn2[:, 1:2], in0=gn2[:, 1:2], in1=gnv, op=Op.subtract)
                nc.scalar.activation(out=gn2[:, 1:2], in_=gn2[:, 1:2], func=Act.Sqrt, bias=eps_t, scale=1.0)
                nc.vector.reciprocal(out=gn2[:, 1:2], in_=gn2[:, 1:2])
                for k in range(4):
                    bcp = pgn.tile([128, 2], f32, tag="bcp", name="bcp")
                    nc.tensor.matmul(bcp, bsel_t[:, k * 128:(k + 1) * 128], gn2, start=True, stop=True)
                    sc = gn.tile([128, 1], f32, tag=f"sc{k}", name=f"sc{k}")
                    tcv = gn.tile([128, 1], f32, tag=f"tc{k}", name=f"tc{k}")
                    nc.vector.tensor_tensor(out=sc, in0=bcp[:, 1:2], in1=gam_t[:, k:k + 1], op=Op.mult)
                    nc.vector.tensor_tensor(out=tcv, in0=bcp[:, 0:1], in1=sc, op=Op.mult)
                    nc.vector.tensor_tensor(out=tcv, in0=bet_t[:, k:k + 1], in1=tcv, op=Op.subtract)
                    nc.vector.tensor_scalar(out=xn_t[k], in0=x_t[k], scalar1=sc, scalar2=tcv,
                                            op0=Op.mult, op1=Op.add)

            # ---- QKV ----
            with tc.tile_pool(name="pqkv", bufs=3, space="PSUM") as pqkv:
                # q, k in standard [c, n] layout (o-chunks 0..7 of 3C)
                for oc in range(8):
                    osl = slice(oc * 128, (oc + 1) * 128)
                    for nt in range(2):
                        nsl = slice(nt * 512, (nt + 1) * 512)
                        ps = pqkv.tile([128, 512], f32, tag="qkv", name="qkv")
                        nc.tensor.matmul(ps, bq_t[0:1, osl], ones_t, start=True, stop=False)
                        for k in range(4):
                            nc.tensor.matmul(ps, wq_t[k][:, osl], xn_t[k][:, nsl],
                                             start=False, stop=(k == 3))
                        nc.vector.tensor_copy(out=qk_t[oc][:, nsl], in_=ps)
                # v^T in [m, c] layout (m-chunks 0..7), bias broadcast along m
                for mc in range(8):
                    msl = slice(mc * 128, (mc + 1) * 128)
                    ps = pqkv.tile([128, 512], f32, tag="qkv", name="qkv")
                    nc.tensor.matmul(ps, ones_t[0:1, 0:128], bq_t[0:1, 2 * C:3 * C], start=True, stop=False)
                    for k in range(4):
                        nc.tensor.matmul(ps, xn_t[k][:, msl], wq_t[k][:, 2 * C:3 * C],
                                         start=False, stop=(k == 3))
                    nc.vector.tensor_copy(
                        out=vp_t[mc][:, :, 0:HD],
                        in_=ps.rearrange("p (h c) -> p h c", h=NH),
                    )

            # ---- attention ----
            with (
                tc.tile_pool(name="att", bufs=2) as att,
                tc.tile_pool(name="pS", bufs=2, space="PSUM") as pS,
                tc.tile_pool(name="pO", bufs=2, space="PSUM") as pO,
            ):
                for hp in range(4):
                    q_t = qk_t[hp]
                    k_t = qk_t[4 + hp]
                    for nt in range(2):
                        nsl = slice(nt * 512, (nt + 1) * 512)
                        outA = pO.tile([HD + 1, 512], f32, tag="outA", name="outA")
                        outB = pO.tile([HD + 1, 512], f32, tag="outB", name="outB")
                        for mc in range(8):
                            msl = slice(mc * 128, (mc + 1) * 128)
                            Sps = pS.tile([128, 1024], f32, tag="S", name="S")
                            nc.tensor.matmul(Sps[:, 0:512], k_t[0:64, msl], q_t[0:64, nsl],
                                             start=True, stop=True)
                            nc.tensor.matmul(Sps[:, 512:1024], k_t[64:128, msl], q_t[64:128, nsl],
                                             start=True, stop=True, tile_position=(64, 0))
                            ex = att.tile([128, 1024], bf16, tag="ex", name="ex")
                            nc.scalar.activation(out=ex, in_=Sps, func=Act.Exp, scale=0.125)
                            nc.tensor.matmul(outA, vp_t[mc][:, 2 * hp, :], ex[:, 0:512],
                                             start=(mc == 0), stop=(mc == 7))
                            nc.tensor.matmul(outB, vp_t[mc][:, 2 * hp + 1, :], ex[:, 512:1024],
                                             start=(mc == 0), stop=(mc == 7))
                        # normalize: row 64 of outA/outB hold the softmax denominators
                        rcA = att.tile([65, 512], f32, tag="rcA", name="rcA")
                        rcB = att.tile([65, 512], f32, tag="rcB", name="rcB")
                        nc.vector.reciprocal(out=rcA[64:65, :], in_=outA[64:65, :])
                        nc.vector.reciprocal(out=rcB[64:65, :], in_=outB[64:65, :])
                        rc2 = att.tile([1, 1024], f32, tag="rc2", name="rc2")
                        nc.gpsimd.dma_start(out=rc2[0:1, 0:512], in_=rcA[64:65, :])
                        nc.gpsimd.dma_start(out=rc2[0:1, 512:1024], in_=rcB[64:65, :])
                        bc2 = att.tile([64, 1024], f32, tag="bc2", name="bc2")
                        nc.gpsimd.partition_broadcast(bc2, rc2[0:1, :])
                        nc.vector.tensor_tensor(out=out_t[hp][0:64, nsl], in0=outA[0:64, :],
                                                in1=bc2[:, 0:512], op=Op.mult)
                        stagB = att.tile([64, 512], bf16, tag="stagB", name="stagB")
                        nc.vector.tensor_tensor(out=stagB, in0=outB[0:64, :],
                                                in1=bc2[:, 512:1024], op=Op.mult)
                        nc.gpsimd.dma_start(out=out_t[hp][64:128, nsl], in_=stagB)

            # ---- proj + residual ----
            with tc.tile_pool(name="pprj", bufs=3, space="PSUM") as pprj:
                for oc in range(4):
                    osl = slice(oc * 128, (oc + 1) * 128)
                    for nt in range(2):
                        nsl = slice(nt * 512, (nt + 1) * 512)
                        ps = pprj.tile([128, 512], f32, tag="prj", name="prj")
                        nc.tensor.matmul(ps, bp_t[0:1, osl], ones_t, start=True, stop=False)
                        for k in range(4):
                            nc.tensor.matmul(ps, wp_t[k][:, osl], out_t[k][:, nsl],
                                             start=False, stop=(k == 3))
                        nc.vector.tensor_tensor(out=y_t[oc][:, nsl], in0=ps, in1=x_t[oc][:, nsl],
                                                op=Op.add)
                for oc in range(4):
                    nc.sync.dma_start(out=y_d[oc * 128:(oc + 1) * 128, :], in_=y_t[oc])

    nc.compile()
    return nc


def _host_inputs(x, gamma, beta, w_qkv, b_qkv, w_proj, b_proj):
    import ml_dtypes
    f = np.float32
    bf = ml_dtypes.bfloat16
    xb = np.ascontiguousarray(np.asarray(x, f).reshape(NCORES, C, N))
    wqkvT = np.ascontiguousarray(np.asarray(w_qkv, f).T.astype(bf))     # [C, 3C]
    bq = np.ascontiguousarray(np.asarray(b_qkv, f)[None, :].astype(bf))
    wprojT = np.ascontiguousarray(np.asarray(w_proj, f).T.astype(bf))   # [C, C]
    bp = np.ascontiguousarray(np.asarray(b_proj, f)[None, :].astype(bf))
    gT = np.ascontiguousarray(np.asarray(gamma, f).reshape(4, 128).T)  # [128, 4]
    bT = np.ascontiguousarray(np.asarray(beta, f).reshape(4, 128).T)
    gsel = np.zeros((128, 4, NG), f)
    bsel = np.zeros((NG, C), f)
    for k in range(4):
        for p in range(128):
            g = 8 * k + p // 16
            gsel[p, k, g] = 1.0 / 16.0
            bsel[g, k * 128 + p] = 1.0
    onesr = np.ones((1, 512), bf)
    onescol = np.ones((128, NH), bf)
    shared = {"wqkvT": wqkvT, "bqkv": bq, "wprojT": wprojT, "bproj": bp,
              "gammaT": gT, "betaT": bT, "gsel": gsel, "bsel": bsel,
              "onesr": onesr, "onescol": onescol}
    return [dict(shared, x=xb[i]) for i in range(NCORES)]


def run(inputs, trace=False, **kwargs):
    from concourse.bass_utils import run_bass_kernel_spmd
    if "nc" not in _CACHE:
        _CACHE["nc"] = _build_program()
    nc = _CACHE["nc"]
    in_maps = _host_inputs(**inputs)
    res = run_bass_kernel_spmd(nc, in_maps, core_ids=list(range(NCORES)), trace=trace, **kwargs)
    B = inputs["x"].shape[0]
    H = W = 32
    y = np.stack([res.results[i]["y"].reshape(C, H, W) for i in range(NCORES)])
    return y.astype(np.float32), res


def kernel(**inputs):
    y, _ = run(inputs, trace=False)
    return y


# revision 8
# speedup vs baseline: 1.3425x; 1.0683x over previous
"""Trainium2 Bass kernel for nn_AttentionBlock (GroupNorm + MHSA + proj + residual).

Data-parallel over batch: B=8 batch elements -> 8 NeuronCores, one each.
Per core (all in fp32, matmuls in fp32r):
  x_b [C=512, N=1024]
  group_norm (32 groups of 16 channels)
  qkv = W_qkv @ x_norm + b  (q,k in [c,n] layout; v computed directly transposed [n,c])
  per head h (8 heads, hd=64): S^T = k_h^T q_h  [m,n]; P = exp(S^T/8);
  out_h = v_h @ P / colsum (colsum via ones-column appended to v^T -> M=65 matmul)
  y = x + W_proj @ out + b_proj
"""

import numpy as np

C = 512
N = 1024  # H*W
NH = 8
HD = 64
NG = 32
EPS = 1e-5
NCORES = 8

_CACHE = {}


def _build_program():
    import concourse.bass as bass  # noqa: F401
    import concourse.mybir as mybir
    import concourse.tile as tile
    from concourse import bacc

    f32 = mybir.dt.float32
    bf16 = mybir.dt.bfloat16
    Act = mybir.ActivationFunctionType
    from concourse.alu_op_type import AluOpType as Op

    nc = bacc.Bacc("TRN2", target_bir_lowering=False, debug=False, num_devices=NCORES)

    # DRAM parameters (per-core shapes; same weights on all cores, x sharded by batch)
    x_d = nc.dram_tensor("x", [C, N], f32, kind="ExternalInput")
    wqkvT_d = nc.dram_tensor("wqkvT", [C, 3 * C], bf16, kind="ExternalInput")
    bqkv_d = nc.dram_tensor("bqkv", [1, 3 * C], bf16, kind="ExternalInput")
    wprojT_d = nc.dram_tensor("wprojT", [C, C], bf16, kind="ExternalInput")
    bproj_d = nc.dram_tensor("bproj", [1, C], bf16, kind="ExternalInput")
    gammaT_d = nc.dram_tensor("gammaT", [128, 4], f32, kind="ExternalInput")
    betaT_d = nc.dram_tensor("betaT", [128, 4], f32, kind="ExternalInput")
    gsel_d = nc.dram_tensor("gsel", [128, 4, NG], f32, kind="ExternalInput")
    bsel_d = nc.dram_tensor("bsel", [NG, C], f32, kind="ExternalInput")
    onesr_d = nc.dram_tensor("onesr", [1, 512], bf16, kind="ExternalInput")
    onescol_d = nc.dram_tensor("onescol", [128, NH], bf16, kind="ExternalInput")
    y_d = nc.dram_tensor("y", [C, N], f32, kind="ExternalOutput")

    with tile.TileContext(nc) as tc:
        with tc.tile_pool(name="mem", bufs=1) as mem:
            # ---- persistent tiles ----
            x_t = [mem.tile([128, N], f32, tag=f"x{k}", name=f"x{k}") for k in range(4)]
            xn_t = [mem.tile([128, N], bf16, tag=f"xn{k}", name=f"xn{k}") for k in range(4)]
            wq_t = [mem.tile([128, 3 * C], bf16, tag=f"wq{k}", name=f"wq{k}") for k in range(4)]
            wp_t = [mem.tile([128, C], bf16, tag=f"wp{k}", name=f"wp{k}") for k in range(4)]
            bq_t = mem.tile([1, 3 * C], bf16, tag="bq", name="bq")
            bp_t = mem.tile([1, C], bf16, tag="bp", name="bp")
            gam_t = mem.tile([128, 4], f32, tag="gam", name="gam")
            bet_t = mem.tile([128, 4], f32, tag="bet", name="bet")
            gsel_t = mem.tile([128, 4, NG], f32, tag="gsel", name="gsel")
            bsel_t = mem.tile([NG, C], f32, tag="bsel", name="bsel")
            ones_t = mem.tile([1, 512], bf16, tag="ones", name="ones")
            qk_t = [mem.tile([128, N], bf16, tag=f"qk{oc}", name=f"qk{oc}") for oc in range(8)]
            # v^T with a ones column appended per head: [128, 8 heads, 65]
            vp_t = [mem.tile([128, NH, HD + 1], bf16, tag=f"vp{mc}", name=f"vp{mc}") for mc in range(8)]
            out_t = [mem.tile([128, N], bf16, tag=f"out{hp}", name=f"out{hp}") for hp in range(4)]
            y_t = [mem.tile([128, N], f32, tag=f"y{k}", name=f"y{k}") for k in range(4)]

            for k in range(4):
                nc.sync.dma_start(out=x_t[k], in_=x_d[k * 128:(k + 1) * 128, :])
            nc.gpsimd.dma_start(out=gam_t, in_=gammaT_d[:])
            nc.gpsimd.dma_start(out=bet_t, in_=betaT_d[:])
            nc.gpsimd.dma_start(out=gsel_t, in_=gsel_d[:])
            nc.gpsimd.dma_start(out=bsel_t, in_=bsel_d[:])
            nc.gpsimd.dma_start(out=ones_t, in_=onesr_d[:])
            nc.gpsimd.dma_start(out=bq_t, in_=bqkv_d[:])
            nc.gpsimd.dma_start(out=bp_t, in_=bproj_d[:])
            for k in range(4):
                nc.sync.dma_start(out=wq_t[k], in_=wqkvT_d[k * 128:(k + 1) * 128, :])
            for k in range(4):
                nc.sync.dma_start(out=wp_t[k], in_=wprojT_d[k * 128:(k + 1) * 128, :])
            for mc in range(8):
                nc.gpsimd.dma_start(out=vp_t[mc][:, :, HD:HD + 1], in_=onescol_d[:, :, None])

            # ---- group norm ----
            with (
                tc.tile_pool(name="gn", bufs=1) as gn,
                tc.tile_pool(name="pgn", bufs=2, space="PSUM") as pgn,
            ):
                s2_t = []
                for k in range(4):
                    st = gn.tile([128, 2, 6], f32, tag=f"st{k}", name=f"st{k}")
                    for j in range(2):
                        nc.vector.bn_stats(out=st[:, j, :], in_=x_t[k][:, j * 512:(j + 1) * 512])
                    mv = gn.tile([128, 2], f32, tag=f"mv{k}", name=f"mv{k}")
                    nc.vector.bn_aggr(out=mv, in_=st)
                    s2 = gn.tile([128, 2], f32, tag=f"s2{k}", name=f"s2{k}")
                    nc.vector.tensor_copy(out=s2[:, 0:1], in_=mv[:, 0:1])
                    nc.vector.tensor_tensor(out=s2[:, 1:2], in0=mv[:, 0:1], in1=mv[:, 0:1], op=Op.mult)
                    nc.vector.tensor_tensor(out=s2[:, 1:2], in0=s2[:, 1:2], in1=mv[:, 1:2], op=Op.add)
                    s2_t.append(s2)
                mvps = pgn.tile([NG, 2], f32, tag="mvps", name="mvps")
                for k in range(4):
                    nc.tensor.matmul(mvps, gsel_t[:, k, :], s2_t[k], start=(k == 0), stop=(k == 3))
                gn2 = gn.tile([NG, 2], f32, tag="gn2", name="gn2")
                eps_t = gn.tile([NG, 1], f32, tag="eps", name="eps")
                nc.vector.memset(eps_t, EPS)
                nc.vector.tensor_copy(out=gn2, in_=mvps)
                gnv = gn.tile([NG, 1], f32, tag="gnv", name="gnv")
                nc.vector.tensor_tensor(out=gnv, in0=gn2[:, 0:1], in1=gn2[:, 0:1], op=Op.mult)
                nc.vector.tensor_tensor(out=gn2[:, 1:2], in0=gn2[:, 1:2], in1=gnv, op=Op.subtract)
                nc.scalar.activation(out=gn2[:, 1:2], in_=gn2[:, 1:2], func=Act.Sqrt, bias=eps_t, scale=1.0)
                nc.vector.reciprocal(out=gn2[:, 1:2], in_=gn2[:, 1:2])
                for k in range(4):
                    bcp = pgn.tile([128, 2], f32, tag="bcp", name="bcp")
                    nc.tensor.matmul(bcp, bsel_t[:, k * 128:(k + 1) * 128], gn2, start=True, stop=True)
                    sc = gn.tile([128, 1], f32, tag=f"sc{k}", name=f"sc{k}")
                    tcv = gn.tile([128, 1], f32, tag=f"tc{k}", name=f"tc{k}")
                    nc.vector.tensor_tensor(out=sc, in0=bcp[:, 1:2], in1=gam_t[:, k:k + 1], op=Op.mult)
                    nc.vector.tensor_tensor(out=tcv, in0=bcp[:, 0:1], in1=sc, op=Op.mult)
                    nc.vector.tensor_tensor(out=tcv, in0=bet_t[:, k:k + 1], in1=tcv, op=Op.subtract)
                    nc.vector.tensor_scalar(out=xn_t[k], in0=x_t[k], scalar1=sc, scalar2=tcv,
                                            op0=Op.mult, op1=Op.add)

            # ---- QKV ----
            with tc.tile_pool(name="pqkv", bufs=3, space="PSUM") as pqkv:
                # q, k in standard [c, n] layout (o-chunks 0..7 of 3C)
                for oc in range(8):
                    osl = slice(oc * 128, (oc + 1) * 128)
                    for nt in range(2):
                        nsl = slice(nt * 512, (nt + 1) * 512)
                        ps = pqkv.tile([128, 512], f32, tag="qkv", name="qkv")
                        nc.tensor.matmul(ps, bq_t[0:1, osl], ones_t, start=True, stop=False)
                        for k in range(4):
                            nc.tensor.matmul(ps, wq_t[k][:, osl], xn_t[k][:, nsl],
                                             start=False, stop=(k == 3))
                        nc.vector.tensor_copy(out=qk_t[oc][:, nsl], in_=ps)
                # v^T in [m, c] layout (m-chunks 0..7), bias broadcast along m
                for mc in range(8):
                    msl = slice(mc * 128, (mc + 1) * 128)
                    ps = pqkv.tile([128, 512], f32, tag="qkv", name="qkv")
                    nc.tensor.matmul(ps, ones_t[0:1, 0:128], bq_t[0:1, 2 * C:3 * C], start=True, stop=False)
                    for k in range(4):
                        nc.tensor.matmul(ps, xn_t[k][:, msl], wq_t[k][:, 2 * C:3 * C],
                                         start=False, stop=(k == 3))
                    nc.vector.tensor_copy(
                        out=vp_t[mc][:, :, 0:HD],
                        in_=ps.rearrange("p (h c) -> p h c", h=NH),
                    )

            # ---- attention (nt-outer so proj of nt=0 overlaps attention of nt=1) ----
            with (
                tc.tile_pool(name="att", bufs=2) as att,
                tc.tile_pool(name="pS", bufs=2, space="PSUM") as pS,
                tc.tile_pool(name="pO", bufs=2, space="PSUM") as pO,
            ):
                for nt in range(2):
                    nsl = slice(nt * 512, (nt + 1) * 512)
                    for hp in range(4):
                        q_t = qk_t[hp]
                        k_t = qk_t[4 + hp]
                        outA = pO.tile([HD + 1, 512], f32, tag="outA", name="outA")
                        outB = pO.tile([HD + 1, 512], f32, tag="outB", name="outB")
                        for mc in range(8):
                            msl = slice(mc * 128, (mc + 1) * 128)
                            Sps = pS.tile([128, 1024], f32, tag="S", name="S")
                            nc.tensor.matmul(Sps[:, 0:512], k_t[0:64, msl], q_t[0:64, nsl],
                                             start=True, stop=True)
                            nc.tensor.matmul(Sps[:, 512:1024], k_t[64:128, msl], q_t[64:128, nsl],
                                             start=True, stop=True, tile_position=(64, 0))
                            ex = att.tile([128, 1024], bf16, tag="ex", name="ex")
                            nc.scalar.activation(out=ex, in_=Sps, func=Act.Exp, scale=0.125)
                            nc.tensor.matmul(outA, vp_t[mc][:, 2 * hp, :], ex[:, 0:512],
                                             start=(mc == 0), stop=(mc == 7))
                            nc.tensor.matmul(outB, vp_t[mc][:, 2 * hp + 1, :], ex[:, 512:1024],
                                             start=(mc == 0), stop=(mc == 7))
                        # normalize: row 64 of outA/outB hold the softmax denominators
                        csA = att.tile([65, 512], f32, tag="csA", name="csA")
                        csB = att.tile([65, 512], f32, tag="csB", name="csB")
                        nc.vector.tensor_copy(out=csA[64:65, :], in_=outA[64:65, :])
                        nc.vector.tensor_copy(out=csB[64:65, :], in_=outB[64:65, :])
                        rc2 = att.tile([1, 1024], f32, tag="rc2", name="rc2")
                        nc.gpsimd.dma_start(out=rc2[0:1, 0:512], in_=csA[64:65, :])
                        nc.gpsimd.dma_start(out=rc2[0:1, 512:1024], in_=csB[64:65, :])
                        rc2b = att.tile([1, 1024], f32, tag="rc2b", name="rc2b")
                        nc.vector.reciprocal_approx_fast(out=rc2b[0:1, :], in_=rc2[0:1, :])
                        bc2 = att.tile([64, 1024], f32, tag="bc2", name="bc2")
                        nc.gpsimd.partition_broadcast(bc2, rc2b[0:1, :])
                        nc.vector.tensor_tensor(out=out_t[hp][0:64, nsl], in0=outA[0:64, :],
                                                in1=bc2[:, 0:512], op=Op.mult)
                        stagB = att.tile([64, 512], bf16, tag="stagB", name="stagB")
                        nc.vector.tensor_tensor(out=stagB, in0=outB[0:64, :],
                                                in1=bc2[:, 512:1024], op=Op.mult)
                        nc.gpsimd.dma_start(out=out_t[hp][64:128, nsl], in_=stagB)
                    # ---- proj + residual for this nt (psum slots shared with S tag) ----
                    for oc in range(4):
                        osl = slice(oc * 128, (oc + 1) * 128)
                        ps = pS.tile([128, 1024], f32, tag="S", name="Sprj")
                        nc.tensor.matmul(ps[:, 0:512], bp_t[0:1, osl], ones_t, start=True, stop=False)
                        for k in range(4):
                            nc.tensor.matmul(ps[:, 0:512], wp_t[k][:, osl], out_t[k][:, nsl],
                                             start=False, stop=(k == 3))
                        nc.vector.tensor_tensor(out=y_t[oc][:, nsl], in0=ps[:, 0:512],
                                                in1=x_t[oc][:, nsl], op=Op.add)
                        nc.sync.dma_start(out=y_d[oc * 128:(oc + 1) * 128, nsl], in_=y_t[oc][:, nsl])

    nc.compile()
    return nc


def _host_inputs(x, gamma, beta, w_qkv, b_qkv, w_proj, b_proj):
    import ml_dtypes
    f = np.float32
    bf = ml_dtypes.bfloat16
    xb = np.ascontiguousarray(np.asarray(x, f).reshape(NCORES, C, N))
    wqkvT = np.ascontiguousarray(np.asarray(w_qkv, f).T.astype(bf))     # [C, 3C]
    bq = np.ascontiguousarray(np.asarray(b_qkv, f)[None, :].astype(bf))
    wprojT = np.ascontiguousarray(np.asarray(w_proj, f).T.astype(bf))   # [C, C]
    bp = np.ascontiguousarray(np.asarray(b_proj, f)[None, :].astype(bf))
    gT = np.ascontiguousarray(np.asarray(gamma, f).reshape(4, 128).T)  # [128, 4]
    bT = np.ascontiguousarray(np.asarray(beta, f).reshape(4, 128).T)
    gsel = np.zeros((128, 4, NG), f)
    bsel = np.zeros((NG, C), f)
    for k in range(4):
        for p in range(128):
            g = 8 * k + p // 16
            gsel[p, k, g] = 1.0 / 16.0
            bsel[g, k * 128 + p] = 1.0
    onesr = np.ones((1, 512), bf)
    onescol = np.ones((128, NH), bf)
    shared = {"wqkvT": wqkvT, "bqkv": bq, "wprojT": wprojT, "bproj": bp,
              "gammaT": gT, "betaT": bT, "gsel": gsel, "bsel": bsel,
              "onesr": onesr, "onescol": onescol}
    return [dict(shared, x=xb[i]) for i in range(NCORES)]


def run(inputs, trace=False, **kwargs):
    from concourse.bass_utils import run_bass_kernel_spmd
    if "nc" not in _CACHE:
        _CACHE["nc"] = _build_program()
    nc = _CACHE["nc"]
    in_maps = _host_inputs(**inputs)
    res = run_bass_kernel_spmd(nc, in_maps, core_ids=list(range(NCORES)), trace=trace, **kwargs)
    B = inputs["x"].shape[0]
    H = W = 32
    y = np.stack([res.results[i]["y"].reshape(C, H, W) for i in range(NCORES)])
    return y.astype(np.float32), res


def kernel(**inputs):
    y, _ = run(inputs, trace=False)
    return y
